# revision 10
# baseline (speedup 1.0000x reference)
"""Trainium2 Bass kernel for the blended-MoE actor network.

Math: reference computes, per sample,
    g1 = relu(bw1 @ s + bb1); g2 = relu(bw2 @ g1 + bb2)
    c  = softmax(bwo @ g2 + bbo)            # 2 experts
    h1 = relu(blend(W1_e, s)); h2 = relu(blend(W2_e, h1))
    mu = tanh(blend(Wm_e, h2))
with blend(W_e, x) = sum_e c_e (W_e x + b_e).

Since NE=2 and c0+c1=1:  c0 = sigmoid((bwo[0]-bwo[1]) @ g2 + dbo)  and
    blend(W_e, x) = W_1 x + b_1 + c0 * (dW x + db),  dW = W_0-W_1.
The c0 * (dW x) term is computed by scaling the matmul INPUT per-sample
(x_c = C0 .* x) so both expert contributions accumulate into one PSUM
group. Rank-1 bias terms c0*db ride along: an appended ones-row on the
states makes row 376 of the scaled states equal c0 (feeding the db
column of the augmented diff weights), and for later layers K=1 matmuls
against the c0 row add c0*db directly.

Layout: activations are [features, batch] on-chip (host pre-transposes
states and appends a ones row). Matmul batch tile N=512 (one PSUM
bank); element-wise work and DMA run on 2048-wide super-tiles to
amortize per-instruction overheads. The router logit-diff matmul uses
wd replicated across 128 output columns so its PSUM output holds the
logit diff in every partition row: one sigmoid per 512 columns yields
the broadcast C0 tile for free.

Sharding: pure data parallel over 8 cores (batch 65536 -> 8 x 8192).
"""

import ml_dtypes
import numpy as np

import concourse.bass as bass
import concourse.mybir as mybir
import concourse.tile as tile
from concourse import bacc
from concourse.bass_utils import run_bass_kernel_spmd

N_CORES = 8
B = 65536
BS = B // N_CORES  # 8192 per core
NI = 376  # state features
NIA = NI + 1  # + ones row
NA = 17  # actions
L1 = 256
L2 = 128
BH = 128  # blending hidden
NT = 512  # matmul batch tile (free dim, one PSUM bank)
ST = 2048  # super-tile for elementwise/DMA
T = BS // ST  # 4 super-tiles
SUB = ST // NT  # 4 matmul tiles per super-tile

F32 = mybir.dt.float32
# Storage/compute dtype. bf16: matmul streams 1 col/cycle at 2.4 GHz
# (f32r measures 2 cyc/col), DVE tensor_tensor gets 2x mode, DMA halves.
DT = mybir.dt.bfloat16
DT_NP = ml_dtypes.bfloat16

AF = mybir.ActivationFunctionType
ALU = mybir.AluOpType

KCH = ((0, 128), (128, 256), (256, NIA))  # K chunks of the state dim


# ---------------------------------------------------------------- weights
# All stationary operands are packed into one [128, WCOLS] host array;
# each lhsT is a column slice [0:K, off:off+M]. Rows >= K are zero.


class _Pack:
    def __init__(self):
        self.cols = []
        self.off = 0

    def add(self, arr):  # arr [K, M] -> returns (off, K, M)
        k, m = arr.shape
        assert k <= 128
        a = np.zeros((128, m), np.float32)
        a[:k] = arr
        off = self.off
        self.cols.append(a)
        self.off += m
        return (off, k, m)

    def data(self):
        return np.concatenate(self.cols, axis=1)


def _prep_weights(p, bw1, bb1, bw2, bb2, bwo, bbo, ew1, eb1, ew2, eb2, ewm, ebm):
    d = {}
    # blend L1: lhsT [377, 128] (= [bw1.T; bb1]) in 3 K-chunks
    w1a = np.concatenate([bw1.T, bb1[None, :]], axis=0)
    d["bl1"] = [p.add(w1a[k0:k1]) for k0, k1 in KCH]
    d["bl2"] = [p.add(bw2.T)]
    d["bb2"] = p.add(bb2[:, None])
    # router logit diff, replicated to 128 output columns
    wd = (bwo[0] - bwo[1])[:, None]
    d["wd"] = [p.add(np.repeat(wd, 128, axis=1))]
    d["bd"] = p.add(np.full((128, 1), bbo[0] - bbo[1], np.float32))
    # expert L1: base = expert1, diff = expert0 - expert1; bias rows appended
    e1b = np.concatenate([ew1[1].T, eb1[1][None, :]], axis=0)  # [377, 256]
    e1d = np.concatenate([(ew1[0] - ew1[1]).T, (eb1[0] - eb1[1])[None, :]], axis=0)
    d["e1b"] = [[p.add(e1b[k0:k1, m : m + 128]) for k0, k1 in KCH] for m in (0, 128)]
    d["e1d"] = [[p.add(e1d[k0:k1, m : m + 128]) for k0, k1 in KCH] for m in (0, 128)]
    # expert L2
    e2b = ew2[1].T  # [256, 128]
    e2d = (ew2[0] - ew2[1]).T
    d["e2b"] = [p.add(e2b[0:128]), p.add(e2b[128:256])]
    d["e2d"] = [p.add(e2d[0:128]), p.add(e2d[128:256])]
    d["db2"] = [p.add((eb2[0] - eb2[1])[None, :])]  # [1, 128]
    d["b2"] = p.add(eb2[1][:, None])
    # expert out
    d["emb"] = [p.add(ewm[1].T)]  # [128, 17]
    d["emd"] = [p.add((ewm[0] - ewm[1]).T)]
    d["dbm"] = [p.add((ebm[0] - ebm[1])[None, :])]  # [1, 17]
    d["bm"] = p.add(ebm[1][:, None])
    return d


# ---------------------------------------------------------------- kernel


def _build(wd, wcols):
    nc = bacc.Bacc("TRN2", target_bir_lowering=False, debug=False,
                   num_devices=N_CORES)
    xs = nc.declare_dram_parameter("xs", [NIA, BS], DT, isOutput=False)
    wk = nc.declare_dram_parameter("wk", [128, wcols], DT, isOutput=False)
    out = nc.declare_dram_parameter("out", [NA, BS], F32, isOutput=True)

    with tile.TileContext(nc) as tc:
        with (
            tc.tile_pool(name="wpool", bufs=1) as wpool,
            tc.tile_pool(name="spool", bufs=2) as spool,
            tc.tile_pool(name="apool", bufs=2) as apool,
            tc.tile_pool(name="opool", bufs=2) as opool,
            tc.tile_pool(name="psum", bufs=1, space="PSUM") as pp,
        ):
            wkt = wpool.tile([128, wcols], DT)
            nc.sync.dma_start(wkt[:], wk[:])

            def W(desc):
                off, k, m = desc
                return wkt[0:k, off : off + m]

            for j in range(T):
                ss = slice(j * ST, (j + 1) * ST)
                # ---- load state chunks [*, ST]
                s = []
                for ci, (k0, k1) in enumerate(KCH):
                    st = spool.tile([k1 - k0, ST], DT, tag=f"s{ci}")
                    nc.sync.dma_start(st[:], xs[k0:k1, ss])
                    s.append(st)

                c0 = apool.tile([128, ST], DT, tag="c0")
                # ---- blending MLP per matmul tile -> C0 slices
                for u in range(SUB):
                    us = slice(u * NT, (u + 1) * NT)
                    pg1 = pp.tile([BH, NT], F32, tag="g1")
                    for ci in range(3):
                        nc.tensor.matmul(pg1[:], W(wd["bl1"][ci]), s[ci][:, us],
                                         start=(ci == 0), stop=(ci == 2))
                    g1 = apool.tile([BH, NT], DT, tag="g1")
                    nc.vector.tensor_scalar_max(g1[:], pg1[:], 0.0)

                    pg2 = pp.tile([BH, NT], F32, tag="g2")
                    nc.tensor.matmul(pg2[:], W(wd["bl2"][0]), g1[:],
                                     start=True, stop=True)
                    g2 = apool.tile([BH, NT], DT, tag="g2")
                    nc.scalar.activation(g2[:], pg2[:], AF.Relu, bias=W(wd["bb2"]))

                    pd = pp.tile([128, NT], F32, tag="d")
                    nc.tensor.matmul(pd[:], W(wd["wd"][0]), g2[:],
                                     start=True, stop=True)
                    nc.scalar.activation(c0[:, us], pd[:], AF.Sigmoid,
                                         bias=W(wd["bd"]))

                # ---- scaled states, full super-tile width (row 376 of xs is
                # ones -> row 120 of chunk 2 becomes c0, feeding the diff
                # bias column of e1d)
                sc = []
                for ci, (k0, k1) in enumerate(KCH):
                    t = spool.tile([k1 - k0, ST], DT, tag=f"sc{ci}")
                    nc.vector.tensor_mul(t[:], s[ci][:], c0[0 : k1 - k0, :])
                    sc.append(t)

                # ---- expert L1
                h1 = [apool.tile([128, ST], DT, tag=f"h1{m}", name=f"h1{m}") for m in range(2)]
                for u in range(SUB):
                    us = slice(u * NT, (u + 1) * NT)
                    for m in range(2):
                        ph = pp.tile([128, NT], F32, tag=f"h1{m}")
                        for ci in range(3):
                            nc.tensor.matmul(ph[:], W(wd["e1b"][m][ci]),
                                             s[ci][:, us],
                                             start=(ci == 0), stop=False)
                        for ci in range(3):
                            nc.tensor.matmul(ph[:], W(wd["e1d"][m][ci]),
                                             sc[ci][:, us],
                                             start=False, stop=(ci == 2))
                        nc.vector.tensor_scalar_max(h1[m][:, us], ph[:], 0.0)

                # ---- expert L2
                h1c = [spool.tile([128, ST], DT, tag=f"h1c{m}", name=f"h1c{m}") for m in range(2)]
                for m in range(2):
                    nc.vector.tensor_mul(h1c[m][:], h1[m][:], c0[:])
                h2 = apool.tile([128, ST], DT, tag="h2")
                for u in range(SUB):
                    us = slice(u * NT, (u + 1) * NT)
                    ph2 = pp.tile([128, NT], F32, tag="h2")
                    nc.tensor.matmul(ph2[:], W(wd["e2b"][0]), h1[0][:, us],
                                     start=True, stop=False)
                    nc.tensor.matmul(ph2[:], W(wd["e2b"][1]), h1[1][:, us],
                                     start=False, stop=False)
                    nc.tensor.matmul(ph2[:], W(wd["e2d"][0]), h1c[0][:, us],
                                     start=False, stop=False)
                    nc.tensor.matmul(ph2[:], W(wd["e2d"][1]), h1c[1][:, us],
                                     start=False, stop=False)
                    nc.tensor.matmul(ph2[:], W(wd["db2"][0]), c0[0:1, us],
                                     start=False, stop=True)
                    nc.scalar.activation(h2[:, us], ph2[:], AF.Relu,
                                         bias=W(wd["b2"]))

                # ---- expert out
                h2c = spool.tile([128, ST], DT, tag="h2c")
                nc.vector.tensor_mul(h2c[:], h2[:], c0[:])
                mu = opool.tile([NA, ST], F32, tag="mu")
                for u in range(SUB):
                    us = slice(u * NT, (u + 1) * NT)
                    pmu = pp.tile([NA, NT], F32, tag="mu")
                    nc.tensor.matmul(pmu[:], W(wd["emb"][0]), h2[:, us],
                                     start=True, stop=False)
                    nc.tensor.matmul(pmu[:], W(wd["emd"][0]), h2c[:, us],
                                     start=False, stop=False)
                    nc.tensor.matmul(pmu[:], W(wd["dbm"][0]), c0[0:1, us],
                                     start=False, stop=True)
                    nc.scalar.activation(mu[:, us], pmu[:], AF.Tanh,
                                         bias=W(wd["bm"]))
                nc.sync.dma_start(out[:, ss], mu[:])
    nc.finalize()
    return nc


_CACHE = {}


def kernel(**inputs) -> np.ndarray:
    states = np.asarray(inputs["states"], np.float32)
    pack = _Pack()
    wdesc = _prep_weights(
        pack,
        *[
            np.asarray(inputs[k], np.float32)
            for k in ("bw1", "bb1", "bw2", "bb2", "bwo", "bbo",
                      "ew1", "eb1", "ew2", "eb2", "ewm", "ebm")
        ],
    )
    wdata = pack.data().astype(DT_NP)  # [128, wcols]

    if "nc" not in _CACHE:
        _CACHE["nc"] = _build(wdesc, wdata.shape[1])
    nc = _CACHE["nc"]

    in_maps = []
    for c in range(N_CORES):
        shard = states[c * BS : (c + 1) * BS]  # [BS, NI]
        xs = np.empty((NIA, BS), np.float32)
        xs[:NI] = shard.T
        xs[NI] = 1.0
        in_maps.append({"xs": xs.astype(DT_NP), "wk": wdata})

    res = run_bass_kernel_spmd(nc, in_maps, core_ids=list(range(N_CORES)))
    out = np.empty((B, NA), np.float32)
    for c in range(N_CORES):
        out[c * BS : (c + 1) * BS] = res.results[c]["out"].T
    return out


# revision 11
# speedup vs baseline: 1.2552x; 1.2552x over previous
"""Trainium2 Bass kernel for the blended-MoE actor network.

Math: reference computes, per sample,
    g1 = relu(bw1 @ s + bb1); g2 = relu(bw2 @ g1 + bb2)
    c  = softmax(bwo @ g2 + bbo)            # 2 experts
    h1 = relu(blend(W1_e, s)); h2 = relu(blend(W2_e, h1))
    mu = tanh(blend(Wm_e, h2))
with blend(W_e, x) = sum_e c_e (W_e x + b_e).

Since NE=2 and c0+c1=1:  c0 = sigmoid((bwo[0]-bwo[1]) @ g2 + dbo)  and
    blend(W_e, x) = W_1 x + b_1 + c0 * (dW x + db),  dW = W_0-W_1.
The c0 * (dW x) term is computed by scaling the matmul INPUT per-sample
(x_c = C0 .* x) so both expert contributions accumulate into one PSUM
group. Rank-1 bias terms c0*db ride along: an appended ones-row on the
states makes row 376 of the scaled states equal c0 (feeding the db
column of the augmented diff weights), and for later layers K=1 matmuls
against the c0 row add c0*db directly.

Layout: activations are [features, batch] on-chip (host pre-transposes
states and appends a ones row); batch tiles of N=512 (one PSUM bank per
matmul). The router logit-diff matmul uses wd replicated across 128
output columns so its PSUM output holds the logit diff in every
partition row: one sigmoid yields the broadcast C0 tile for free.

The per-tile dataflow is a serial chain (blend MLP -> C0 -> scaled
inputs -> expert layers), so instructions are emitted in an explicit
software-pipelined order across batch tiles -- the TensorEngine stream
interleaves expert layers of tiles t, t-1, t-2 with the blend MLP of
tile t+2, keeping the PE free of cross-engine round-trip stalls.

Sharding: pure data parallel over 8 cores (batch 65536 -> 8 x 8192).
"""

import ml_dtypes
import numpy as np

import concourse.bass as bass
import concourse.mybir as mybir
import concourse.tile as tile
from concourse import bacc
from concourse.bass_utils import run_bass_kernel_spmd

N_CORES = 8
B = 65536
BS = B // N_CORES  # 8192 per core
NI = 376  # state features
NIA = NI + 1  # + ones row
NA = 17  # actions
BH = 128  # blending hidden
NT = 512  # batch tile (matmul free dim, one PSUM bank)
T = BS // NT  # 16 tiles per core

F32 = mybir.dt.float32
# bf16 compute: matmul streams 1 col/cycle warm (f32r measures 2), DVE
# tensor_tensor gets 2x mode, DMA bytes halve. rel err ~8e-3 << 2e-2.
DT = mybir.dt.bfloat16
DT_NP = ml_dtypes.bfloat16

AF = mybir.ActivationFunctionType
KCH = ((0, 128), (128, 256), (256, NIA))  # K chunks of the state dim


# ---------------------------------------------------------------- weights
# All stationary operands are packed into one [128, WCOLS] host array;
# each lhsT is a column slice [0:K, off:off+M]. Rows >= K are zero.


class _Pack:
    def __init__(self):
        self.cols = []
        self.off = 0

    def add(self, arr):  # arr [K, M] -> returns (off, K, M)
        k, m = arr.shape
        assert k <= 128
        a = np.zeros((128, m), np.float32)
        a[:k] = arr
        off = self.off
        self.cols.append(a)
        self.off += m
        return (off, k, m)

    def data(self):
        return np.concatenate(self.cols, axis=1)


def _prep_weights(p, bw1, bb1, bw2, bb2, bwo, bbo, ew1, eb1, ew2, eb2, ewm, ebm):
    d = {}
    # blend L1: lhsT [377, 128] (= [bw1.T; bb1]) in 3 K-chunks
    w1a = np.concatenate([bw1.T, bb1[None, :]], axis=0)
    d["bl1"] = [p.add(w1a[k0:k1]) for k0, k1 in KCH]
    d["bl2"] = [p.add(bw2.T)]
    d["bb2"] = p.add(bb2[:, None])
    # router logit diff, replicated to 128 output columns
    wd = (bwo[0] - bwo[1])[:, None]
    d["wd"] = [p.add(np.repeat(wd, 128, axis=1))]
    d["bd"] = p.add(np.full((128, 1), bbo[0] - bbo[1], np.float32))
    # expert L1: base = expert1, diff = expert0 - expert1; bias rows appended
    e1b = np.concatenate([ew1[1].T, eb1[1][None, :]], axis=0)  # [377, 256]
    e1d = np.concatenate([(ew1[0] - ew1[1]).T, (eb1[0] - eb1[1])[None, :]], axis=0)
    d["e1b"] = [[p.add(e1b[k0:k1, m : m + 128]) for k0, k1 in KCH] for m in (0, 128)]
    d["e1d"] = [[p.add(e1d[k0:k1, m : m + 128]) for k0, k1 in KCH] for m in (0, 128)]
    # expert L2
    e2b = ew2[1].T  # [256, 128]
    e2d = (ew2[0] - ew2[1]).T
    d["e2b"] = [p.add(e2b[0:128]), p.add(e2b[128:256])]
    d["e2d"] = [p.add(e2d[0:128]), p.add(e2d[128:256])]
    d["db2"] = [p.add((eb2[0] - eb2[1])[None, :])]  # [1, 128]
    d["b2"] = p.add(eb2[1][:, None])
    # expert out
    d["emb"] = [p.add(ewm[1].T)]  # [128, 17]
    d["emd"] = [p.add((ewm[0] - ewm[1]).T)]
    d["dbm"] = [p.add((ebm[0] - ebm[1])[None, :])]  # [1, 17]
    d["bm"] = p.add(ebm[1][:, None])
    return d


# ---------------------------------------------------------------- kernel


def _build(wd, wcols):
    nc = bacc.Bacc("TRN2", target_bir_lowering=False, debug=False,
                   num_devices=N_CORES)
    xs = nc.declare_dram_parameter("xs", [NIA, BS], DT, isOutput=False)
    wk = nc.declare_dram_parameter("wk", [128, wcols], DT, isOutput=False)
    out = nc.declare_dram_parameter("out", [NA, BS], F32, isOutput=True)

    with tile.TileContext(nc) as tc:
        with (
            tc.tile_pool(name="wpool", bufs=1) as wpool,
            tc.tile_pool(name="spool", bufs=5) as spool,
            tc.tile_pool(name="scpool", bufs=3) as scpool,
            tc.tile_pool(name="gpool", bufs=3) as gpool,
            tc.tile_pool(name="cpool", bufs=6) as cpool,
            tc.tile_pool(name="hpool", bufs=3) as hpool,
            tc.tile_pool(name="opool", bufs=2) as opool,
            tc.tile_pool(name="psum", bufs=1, space="PSUM") as pp,
        ):
            wkt = wpool.tile([128, wcols], DT)
            nc.sync.dma_start(wkt[:], wk[:])

            def W(desc):
                off, k, m = desc
                return wkt[0:k, off : off + m]

            # per-tile live tensors, keyed by tile index
            s = {}
            sc = {}
            c0 = {}
            g1 = {}
            g2 = {}
            h1 = {}
            h1c = {}
            h2 = {}
            h2c = {}

            def csl(t):
                return slice(t * NT, (t + 1) * NT)

            def dma_in(t):
                s[t] = []
                for ci, (k0, k1) in enumerate(KCH):
                    st = spool.tile([k1 - k0, NT], DT, tag=f"s{ci}",
                                    name=f"s{ci}_{t}")
                    nc.sync.dma_start(st[:], xs[k0:k1, csl(t)])
                    s[t].append(st)

            def blend_g1(t):
                pg1 = pp.tile([BH, NT], F32, tag="g1", name=f"pg1_{t}")
                for ci in range(3):
                    nc.tensor.matmul(pg1[:], W(wd["bl1"][ci]), s[t][ci][:],
                                     start=(ci == 0), stop=(ci == 2))
                g1[t] = gpool.tile([BH, NT], DT, tag="g1", name=f"g1_{t}")
                nc.vector.tensor_scalar_max(g1[t][:], pg1[:], 0.0)

            def blend_g2(t):
                pg2 = pp.tile([BH, NT], F32, tag="g2", name=f"pg2_{t}")
                nc.tensor.matmul(pg2[:], W(wd["bl2"][0]), g1[t][:],
                                 start=True, stop=True)
                g2[t] = gpool.tile([BH, NT], DT, tag="g2", name=f"g2_{t}")
                nc.scalar.activation(g2[t][:], pg2[:], AF.Relu, bias=W(wd["bb2"]))

            def blend_d(t):
                pd = pp.tile([128, NT], F32, tag="d", name=f"pd_{t}")
                nc.tensor.matmul(pd[:], W(wd["wd"][0]), g2[t][:],
                                 start=True, stop=True)
                c0[t] = cpool.tile([128, NT], DT, tag="c0", name=f"c0_{t}")
                nc.scalar.activation(c0[t][:], pd[:], AF.Sigmoid, bias=W(wd["bd"]))
                # scaled states on GpSimd (SBUF-only) to unload the DVE;
                # row 120 of chunk 2 becomes c0 via the ones-row
                sc[t] = []
                for ci, (k0, k1) in enumerate(KCH):
                    tt = scpool.tile([k1 - k0, NT], DT, tag=f"sc{ci}",
                                     name=f"sc{ci}_{t}")
                    nc.gpsimd.tensor_mul(tt[:], s[t][ci][:], c0[t][0 : k1 - k0, :])
                    sc[t].append(tt)

            def exp_l1(t):
                h1[t] = []
                h1c[t] = []
                for m in range(2):
                    ph = pp.tile([128, NT], F32, tag=f"h1{m}", name=f"ph1{m}_{t}")
                    for ci in range(3):
                        nc.tensor.matmul(ph[:], W(wd["e1b"][m][ci]), s[t][ci][:],
                                         start=(ci == 0), stop=False)
                    for ci in range(3):
                        nc.tensor.matmul(ph[:], W(wd["e1d"][m][ci]), sc[t][ci][:],
                                         start=False, stop=(ci == 2))
                    ht = hpool.tile([128, NT], DT, tag=f"h1{m}", name=f"h1{m}_{t}")
                    nc.vector.tensor_scalar_max(ht[:], ph[:], 0.0)
                    h1[t].append(ht)
                    htc = hpool.tile([128, NT], DT, tag=f"h1c{m}",
                                     name=f"h1c{m}_{t}")
                    nc.vector.tensor_mul(htc[:], ht[:], c0[t][:])
                    h1c[t].append(htc)

            def exp_l2(t):
                ph2 = pp.tile([128, NT], F32, tag="h2", name=f"ph2_{t}")
                nc.tensor.matmul(ph2[:], W(wd["e2b"][0]), h1[t][0][:],
                                 start=True, stop=False)
                nc.tensor.matmul(ph2[:], W(wd["e2b"][1]), h1[t][1][:],
                                 start=False, stop=False)
                nc.tensor.matmul(ph2[:], W(wd["e2d"][0]), h1c[t][0][:],
                                 start=False, stop=False)
                nc.tensor.matmul(ph2[:], W(wd["e2d"][1]), h1c[t][1][:],
                                 start=False, stop=False)
                nc.tensor.matmul(ph2[:], W(wd["db2"][0]), c0[t][0:1, :],
                                 start=False, stop=True)
                h2[t] = hpool.tile([128, NT], DT, tag="h2", name=f"h2_{t}")
                nc.scalar.activation(h2[t][:], ph2[:], AF.Relu, bias=W(wd["b2"]))
                h2c[t] = hpool.tile([128, NT], DT, tag="h2c", name=f"h2c_{t}")
                nc.vector.tensor_mul(h2c[t][:], h2[t][:], c0[t][:])
                # s/sc no longer needed
                del s[t], sc[t], g1[t], g2[t]

            def exp_l3(t):
                pmu = pp.tile([NA, NT], F32, tag="mu", name=f"pmu_{t}")
                nc.tensor.matmul(pmu[:], W(wd["emb"][0]), h2[t][:],
                                 start=True, stop=False)
                nc.tensor.matmul(pmu[:], W(wd["emd"][0]), h2c[t][:],
                                 start=False, stop=False)
                nc.tensor.matmul(pmu[:], W(wd["dbm"][0]), c0[t][0:1, :],
                                 start=False, stop=True)
                mu = opool.tile([NA, NT], F32, tag="mu", name=f"mu_{t}")
                nc.scalar.activation(mu[:], pmu[:], AF.Tanh, bias=W(wd["bm"]))
                nc.sync.dma_start(out[:, csl(t)], mu[:])
                del h1[t], h1c[t], h2[t], h2c[t], c0[t]

            # -------- software-pipelined emission --------
            # prologue: states for tiles 0..2, blend chains for 0 and 1
            # (interleaved to overlap their serial MM->ACT/DVE hops)
            for t in (0, 1, 2):
                dma_in(t)
            blend_g1(0)
            blend_g1(1)
            blend_g2(0)
            blend_g2(1)
            blend_d(0)
            blend_d(1)
            # steady state: iteration t runs L1(t), L2(t-1), L3(t-2) and
            # the blend MLP of t+2 spliced between expert blocks
            for t in range(T):
                if t + 3 < T:
                    dma_in(t + 3)
                if t + 2 < T:
                    blend_g1(t + 2)
                exp_l1(t)
                if t + 2 < T:
                    blend_g2(t + 2)
                if t >= 1:
                    exp_l2(t - 1)
                if t + 2 < T:
                    blend_d(t + 2)
                if t >= 2:
                    exp_l3(t - 2)
            exp_l2(T - 1)
            exp_l3(T - 2)
            exp_l3(T - 1)
    nc.finalize()
    return nc


_CACHE = {}


def kernel(**inputs) -> np.ndarray:
    states = np.asarray(inputs["states"], np.float32)
    pack = _Pack()
    wdesc = _prep_weights(
        pack,
        *[
            np.asarray(inputs[k], np.float32)
            for k in ("bw1", "bb1", "bw2", "bb2", "bwo", "bbo",
                      "ew1", "eb1", "ew2", "eb2", "ewm", "ebm")
        ],
    )
    wdata = pack.data().astype(DT_NP)  # [128, wcols]

    if "nc" not in _CACHE:
        _CACHE["nc"] = _build(wdesc, wdata.shape[1])
    nc = _CACHE["nc"]

    in_maps = []
    for c in range(N_CORES):
        shard = states[c * BS : (c + 1) * BS]  # [BS, NI]
        xs = np.empty((NIA, BS), np.float32)
        xs[:NI] = shard.T
        xs[NI] = 1.0
        in_maps.append({"xs": xs.astype(DT_NP), "wk": wdata})

    res = run_bass_kernel_spmd(nc, in_maps, core_ids=list(range(N_CORES)))
    out = np.empty((B, NA), np.float32)
    for c in range(N_CORES):
        out[c * BS : (c + 1) * BS] = res.results[c]["out"].T
    return out
